# revision 42
# baseline (speedup 1.0000x reference)
"""ConcatenatedLoRALinearSidecarLayer kernel for 8x TRN2 NeuronCores.

Reference computation (per LoRA branch n, then concat over n on the last dim):
    h_n = x @ down_n.T                      # [M, R]
    y_n = (h_n @ up_n.T + bias_n) * (WEIGHT * scales_n)
    out = concat_n(y_n)                     # [M, N*O]

Strategy (final — fp16 DMA path, DMA-byte roofline ~73 MB/core):
  - Data-parallel over tokens M = B*S = 16384 -> 2048 tokens per core.
  - Host-side prep: transpose x to [D, M], fold WEIGHT*scales into up/bias,
    cast x/down/up/bias to fp16 (halves HBM traffic vs fp32; PE accumulates
    in fp32 so rel-err stays ~1e-3).
  - Output y is stored as fp16 and widened to fp32 on the host.
  - Per core, for each 512-token block:
      phase 1:  hT_n[r, t] = sum_d downT_n[d, r] * xT[d, t]   (branch-outer,
                d-chunk inner; dT stationary, xT moving free dim 512)
      phase 2:  y[t, o] = sum_r hT_n[r, t] * upT_n[r, o]
                (hT 128x128 stationary, upT moving free dim 512, two matmuls
                 fill one 2-bank PSUM tile [128, 1024])
      epilogue: PSUM->SBUF split across engines — ScalarE copies 3 of 4
                tiles per branch, DVE does the 4th fused with the bias add,
                then one cheap fp16 SBUF 2x-mode DVE add applies bias to the
                ScalarE-copied region. One 3 MB DMA store per 128 tokens.
  - Next block's x loads are emitted BEFORE this block's y stores so the SP
    HWDGE FIFO never parks a load behind store semaphores.
  - Each block's last-th store is deferred one block so its packets drain in
    the block-transition window where the DMA queue would otherwise run dry;
    the last block stores per branch (1 MB) to shrink the final drain.
  - A dummy-matmul warmup after the dT load un-throttles the PE HAM clock
    gate before the first real matmul; redundant LDWEIGHTS are deduped in a
    BIR post-pass (phase 2 reuses each hT slice for 8 matmuls).
  - All weights (downT, upT, bias) stay resident in SBUF.

Wait-slot legalization: this container's walrus accepts at most 1 sync-wait
per instruction; _legalize_wait_counts splits excess waits onto same-engine
NoOps in the serialized BIR (identical blocking semantics).
"""

from contextlib import ExitStack

import numpy as np

import concourse.bass as bass
import concourse.mybir as mybir
import concourse.tile as tile

WEIGHT = 0.8
N_CORES = 8
B, S, D = 4, 4096, 4096
NL, R, O = 3, 128, 4096
M = B * S                    # 16384 tokens total
T = M // N_CORES             # 2048 tokens per core
NR = NL * R                  # 384
NO = NL * O                  # 12288

P = 128                      # SBUF partitions
TB = 512                     # token block (phase-1 moving free dim)
DO = D // P                  # 32 contraction chunks
DH = DO // 2                 # d-chunks per x half-load
OC = 1024                    # phase-2 PSUM tile free dim (2 banks)
NOC = O // OC                # 4 output chunks per branch
ACT_OC = NOC - 1             # chunks 0..ACT_OC-1 copied by ScalarE

F32 = mybir.dt.float32
F16 = mybir.dt.float16

# Drop redundant Ldweights when consecutive PE matmuls reuse the same
# stationary operand (phase 2 issues 8 matmuls per hT slice).
LDW_DEDUPE = True
# Broadcast bias across partitions with GpSimd instead of a 3 MB DMA.
GPSIMD_BCAST = False
# Number of chunks to split the partition_broadcast into.
BCAST_CHUNKS = 1
# Broadcast bias across partitions with a PE rank-1 matmul (ones x bias_row)
# plus ScalarE PSUM->SBUF copies, all during the DMA ramp — keeps the 3 MB
# of broadcast writes off the SDMA ports.
PE_BCAST = False
# Dummy matmul burst after the dT load so the PE HAM un-throttles (1.2 ->
# 2.4 GHz) before the first real phase-1 matmul. (The PE_BCAST matmuls
# already provide the warmup, so keep this off when PE_BCAST is on.)
PE_WARMUP = True


def build_nc(t_core: int = T) -> bass.Bass:
    assert t_core % TB == 0
    n_tb = t_core // TB

    nc = bass.Bass("TRN2", target_bir_lowering=False, debug=False)

    xT = nc.dram_tensor("xT", [D, t_core], F16, kind="ExternalInput")
    dT = nc.dram_tensor("dT", [D, NR], F16, kind="ExternalInput")
    uT = nc.dram_tensor("uT", [R, NO], F16, kind="ExternalInput")
    bw = nc.dram_tensor("bw", [1, NO], F16, kind="ExternalInput")
    y = nc.dram_tensor("y", [t_core, NO], F16, kind="ExternalOutput")

    with tile.TileContext(nc) as tc, ExitStack() as ctx:
        const = ctx.enter_context(tc.tile_pool(name="const", bufs=1))
        xpool = ctx.enter_context(tc.tile_pool(name="xpool", bufs=3))
        hpool = ctx.enter_context(tc.tile_pool(name="hpool", bufs=2))
        ypool = ctx.enter_context(tc.tile_pool(name="ypool", bufs=9))
        ps_y = ctx.enter_context(tc.tile_pool(name="ps_y", bufs=3, space="PSUM"))
        ps_h = ctx.enter_context(tc.tile_pool(name="ps_h", bufs=2, space="PSUM"))

        # Resident weights: downT first (phase 1 needs it immediately).
        dT_sb = const.tile([P, DO, NR], F16, name="dT_sb")
        nc.sync.dma_start(dT_sb[:], dT.ap().rearrange("(do di) nr -> di do nr", di=P))

        if PE_WARMUP:
            # ~3.3us of garbage matmuls (identical stationary operand, so
            # LDW_DEDUPE collapses the weight loads) to warm the HAM clock
            # gate while the first x block is still in flight.
            warm = ps_h.tile([P, TB], F32, tag="hps", name="warm")
            for _ in range(16):
                nc.tensor.matmul(
                    warm[:, 0:384], dT_sb[:, 0, 0:P], dT_sb[:, 0, 0:384],
                    start=True, stop=True,
                )

        xTr = xT.ap().rearrange("(do di) t -> di do t", di=P)

        def load_x(tb, pieces=2):
            xts = []
            dq = DO // pieces
            for h in range(pieces):
                xt = xpool.tile([P, dq, TB], F16, tag="xt", name=f"xt{tb}_{h}")
                nc.sync.dma_start(
                    xt[:], xTr[:, h * dq:(h + 1) * dq, tb * TB:(tb + 1) * TB]
                )
                xts.append(xt)
            return xts, dq

        bw_sb = const.tile([P, NO], F16, name="bw_sb")
        if PE_BCAST:
            # Bias lands in partition 0 (24 KB DMA, right after dT), then a
            # rank-1 PE matmul (ones x bias_row) fans it out to PSUM and
            # ScalarE copies it to SBUF — all during the DMA ramp. This
            # keeps ~3 MB of broadcast writes off the SDMA ports and doubles
            # as the HAM warmup. bw_row borrows a ypool rotation slot that is
            # long dead before the first real output tile needs it.
            bw_row = ypool.tile([1, NO], F16, tag="ysb", name="bw_row")
            nc.sync.dma_start(bw_row[0:1, :], bw[:, :])
            ones = const.tile([1, P], F16, name="ones")
            nc.vector.memset(ones[:], 1.0)
            for c in range(NO // 512):
                bps = ps_y.tile([P, 512], F32, tag="yps", name=f"bps{c}")
                nc.tensor.matmul(
                    bps[:], ones[0:1, :], bw_row[0:1, c * 512:(c + 1) * 512],
                    start=True, stop=True,
                )
                nc.scalar.copy(bw_sb[:, c * 512:(c + 1) * 512], bps[:])

        xts, xdq = load_x(0)

        uT_sb = const.tile([P, NO], F16, name="uT_sb")
        nc.sync.dma_start(uT_sb[:], uT[:, :])
        if GPSIMD_BCAST:
            nc.sync.dma_start(bw_sb[0:1, :], bw[:, :])
            from concourse import library_config
            nc.gpsimd.load_library(library_config.mlp)
            cw = NO // BCAST_CHUNKS
            for c in range(BCAST_CHUNKS):
                nc.gpsimd.partition_broadcast(
                    bw_sb[:, c * cw:(c + 1) * cw], bw_sb[0:1, c * cw:(c + 1) * cw]
                )
        elif not PE_BCAST:
            nc.sync.dma_start(bw_sb[:], bw.ap().to_broadcast((P, NO)))

        pending_stores = []
        for tb in range(n_tb):
            # Phase 1: hT_n[r, 0:TB], branch-outer so only one PSUM bank live.
            hT = hpool.tile([P, NL, TB], F16, tag="hT", name=f"hT{tb}")
            for n in range(NL):
                hps = ps_h.tile([P, TB], F32, tag="hps", name=f"hps{tb}_{n}")
                for dc in range(DO):
                    nc.tensor.matmul(
                        hps[:],
                        dT_sb[:, dc, n * R:(n + 1) * R],
                        xts[dc // xdq][:, dc % xdq, :],
                        start=(dc == 0),
                        stop=(dc == DO - 1),
                    )
                nc.scalar.copy(hT[:, n, :], hps[:])

            # Prefetch next block's x before this block's y stores hit the
            # SP HWDGE FIFO.
            if tb + 1 < n_tb:
                xts, xdq = load_x(tb + 1)

            # The previous block's deferred th2/th3 stores go out here: their
            # packets land exactly in the block-transition window where the
            # queue supply would otherwise collapse (x prefetched, no
            # epilogue output ready yet).
            for t0p, o0p, ytp in pending_stores:
                nc.sync.dma_start(y[t0p:t0p + P, o0p:o0p + O], ytp[:])
            pending_stores = []

            # Phase 2 + epilogue, per 128-token sub-block. All stores are
            # per-branch 1 MB transfers (measured ~400 GB/s, same as 3 MB).
            for th in range(TB // P):
                t0 = tb * TB + th * P
                for n in range(NL):
                    lhs = hT[:, n, th * P:(th + 1) * P]
                    o0 = n * O
                    yt = ypool.tile([P, O], F16, tag="ysb",
                                    name=f"ysb{tb}_{th}_{n}")
                    for oc in range(NOC):
                        yps = ps_y.tile([P, OC], F32, tag="yps",
                                        name=f"yps{tb}_{th}_{n}_{oc}")
                        for hh in range(2):
                            c0 = oc * OC + hh * 512
                            nc.tensor.matmul(
                                yps[:, hh * 512:(hh + 1) * 512],
                                lhs,
                                uT_sb[:, o0 + c0: o0 + c0 + 512],
                                start=True,
                                stop=True,
                            )
                        dst = yt[:, oc * OC:(oc + 1) * OC]
                        if oc < ACT_OC:
                            nc.scalar.copy(dst, yps[:])
                        else:
                            nc.vector.tensor_add(
                                dst, yps[:],
                                bw_sb[:, o0 + oc * OC: o0 + (oc + 1) * OC],
                            )
                    # Bias for the ScalarE-copied region (fp16 SBUF 2x mode).
                    acn = ACT_OC * OC
                    nc.vector.tensor_add(
                        yt[:, 0:acn],
                        yt[:, 0:acn],
                        bw_sb[:, o0: o0 + acn],
                    )
                    if tb < n_tb - 1 and th >= TB // P - 2:
                        pending_stores.append((t0, o0, yt))
                    else:
                        nc.sync.dma_start(y[t0:t0 + P, o0:o0 + O], yt[:])

    _wrap_to_json_with_wait_split(nc)
    return nc


def _dedupe_ldweights(bir: dict) -> None:
    """Remove Ldweights whose stationary operand is identical to the
    previous Ldweights in the same block (the PE keeps the loaded weights
    until replaced). Only sync-free Ldweights are dropped."""
    import json as _json

    for fn in bir.get("functions", []):
        for blk in fn.get("blocks", []):
            insts = blk.get("instructions", [])
            out = []
            last_w = None
            for inst in insts:
                if inst.get("opcode") == "Ldweights":
                    key = _json.dumps(inst.get("ins"), sort_keys=True)
                    si = inst.get("sync_info") or {}
                    if (key == last_w and not si.get("on_wait")
                            and not si.get("on_update")):
                        continue
                    last_w = key
                out.append(inst)
            blk["instructions"] = out


def _legalize_wait_counts(bir: dict) -> None:
    """Split multi-wait instructions: this walrus accepts only ONE sync-wait
    per instruction. Excess waits move onto NoOps inserted just before the
    instruction on the same engine — identical blocking semantics."""
    n_new = 0
    for fn in bir.get("functions", []):
        for blk in fn.get("blocks", []):
            insts = blk.get("instructions", [])
            out = []
            for inst in insts:
                si = inst.get("sync_info")
                waits = (si or {}).get("on_wait") or []
                if len(waits) > 1:
                    for w in waits[:-1]:
                        nonlocal_name = f"I-waitsplit-{id(inst)}-{n_new}"
                        n_new += 1
                        out.append({
                            "debug": inst.get("debug", 0),
                            "engine": inst["engine"],
                            "ins": [],
                            "name": nonlocal_name,
                            "opcode": "NoOp",
                            "outs": [],
                            "sync_info": {"on_update": [], "on_wait": [w]},
                        })
                    si["on_wait"] = [waits[-1]]
                out.append(inst)
            blk["instructions"] = out


def _wrap_to_json_with_wait_split(nc) -> None:
    import json as _json

    orig = nc.to_json_bytes

    def patched():
        d = _json.loads(orig())
        if LDW_DEDUPE:
            _dedupe_ldweights(d)
        _legalize_wait_counts(d)
        return _json.dumps(d).encode()

    nc.to_json_bytes = patched


def prep_inputs(x, down, up, bias, scales):
    """Host-side marshalling: transpose + fold scales + cast fp16."""
    x = np.asarray(x, dtype=np.float32)
    down = np.asarray(down, dtype=np.float32)
    up = np.asarray(up, dtype=np.float32)
    bias = np.asarray(bias, dtype=np.float32)
    scales = np.asarray(scales, dtype=np.float32)

    ws = (WEIGHT * scales).astype(np.float32)                       # [NL]
    xTf = np.ascontiguousarray(x.reshape(M, D).T).astype(np.float16)
    dTf = np.ascontiguousarray(
        np.transpose(down, (2, 0, 1)).reshape(D, NR)).astype(np.float16)
    uTf = np.ascontiguousarray(
        np.transpose(up * ws[:, None, None], (2, 0, 1)).reshape(R, NO)
    ).astype(np.float16)
    bwf = np.ascontiguousarray(
        (bias * ws[:, None]).reshape(1, NO)).astype(np.float16)

    in_maps = []
    for c in range(N_CORES):
        in_maps.append({
            "xT": np.ascontiguousarray(xTf[:, c * T:(c + 1) * T]),
            "dT": dTf,
            "uT": uTf,
            "bw": bwf,
        })
    return in_maps


_CACHED_NC = None


def kernel(x, down, up, bias, scales):
    global _CACHED_NC
    from concourse.bass_utils import run_bass_kernel_spmd

    in_maps = prep_inputs(x, down, up, bias, scales)
    if _CACHED_NC is None:
        _CACHED_NC = build_nc(T)
    res = run_bass_kernel_spmd(_CACHED_NC, in_maps, core_ids=list(range(N_CORES)))
    out = np.concatenate([r["y"] for r in res.results], axis=0)
    return out.astype(np.float32).reshape(B, S, NO)


# revision 44
# speedup vs baseline: 1.0128x; 1.0128x over previous
"""ConcatenatedLoRALinearSidecarLayer kernel for 8x TRN2 NeuronCores.

Reference computation (per LoRA branch n, then concat over n on the last dim):
    h_n = x @ down_n.T                      # [M, R]
    y_n = (h_n @ up_n.T + bias_n) * (WEIGHT * scales_n)
    out = concat_n(y_n)                     # [M, N*O]

Strategy (v2 — fp16 DMA path, DMA-byte roofline ~73 MB/core):
  - Data-parallel over tokens M = B*S = 16384 -> 2048 tokens per core.
  - Host-side prep: transpose x to [D, M], fold WEIGHT*scales into up/bias,
    cast x/down/up/bias to fp16 (halves HBM traffic vs fp32; PE accumulates
    in fp32 so rel-err stays ~1e-3).
  - Output y is stored as fp16 and widened to fp32 on the host.
  - Per core, for each 512-token block:
      phase 1:  hT_n[r, t] = sum_d downT_n[d, r] * xT[d, t]   (branch-outer,
                d-chunk inner; dT stationary, xT moving free dim 512)
      phase 2:  y[t, o] = sum_r hT_n[r, t] * upT_n[r, o]
                (hT 128x128 stationary, upT moving free dim 512, two matmuls
                 fill one 2-bank PSUM tile [128, 1024])
      epilogue: PSUM->SBUF split across engines — ScalarE copies 3 of 4
                tiles per branch, DVE does the 4th fused with the bias add,
                then one cheap fp16 SBUF 2x-mode DVE add applies bias to the
                ScalarE-copied region. One 3 MB DMA store per 128 tokens.
  - Next block's x loads are emitted BEFORE this block's y stores so the SP
    HWDGE FIFO never parks a load behind store semaphores.
  - All weights (downT, upT, bias) stay resident in SBUF.

Wait-slot legalization: this container's walrus accepts at most 1 sync-wait
per instruction; _legalize_wait_counts splits excess waits onto same-engine
NoOps in the serialized BIR (identical blocking semantics).
"""

from contextlib import ExitStack

import numpy as np

import concourse.bass as bass
import concourse.mybir as mybir
import concourse.tile as tile

WEIGHT = 0.8
N_CORES = 8
B, S, D = 4, 4096, 4096
NL, R, O = 3, 128, 4096
M = B * S                    # 16384 tokens total
T = M // N_CORES             # 2048 tokens per core
NR = NL * R                  # 384
NO = NL * O                  # 12288

P = 128                      # SBUF partitions
TB = 512                     # token block (phase-1 moving free dim)
DO = D // P                  # 32 contraction chunks
DH = DO // 2                 # d-chunks per x half-load
OC = 1024                    # phase-2 PSUM tile free dim (2 banks)
NOC = O // OC                # 4 output chunks per branch
ACT_OC = NOC - 1             # chunks 0..ACT_OC-1 copied by ScalarE

F32 = mybir.dt.float32
F16 = mybir.dt.float16

# Drop redundant Ldweights when consecutive PE matmuls reuse the same
# stationary operand (phase 2 issues 8 matmuls per hT slice).
LDW_DEDUPE = True
# Broadcast bias across partitions with GpSimd instead of a 3 MB DMA.
GPSIMD_BCAST = False
# Number of chunks to split the partition_broadcast into.
BCAST_CHUNKS = 1
# Broadcast bias across partitions with a PE rank-1 matmul (ones x bias_row)
# plus ScalarE PSUM->SBUF copies, all during the DMA ramp — keeps the 3 MB
# of broadcast writes off the SDMA ports.
PE_BCAST = False
# Dummy matmul burst after the dT load so the PE HAM un-throttles (1.2 ->
# 2.4 GHz) before the first real phase-1 matmul. (The PE_BCAST matmuls
# already provide the warmup, so keep this off when PE_BCAST is on.)
PE_WARMUP = True


def build_nc(t_core: int = T) -> bass.Bass:
    assert t_core % TB == 0
    n_tb = t_core // TB

    nc = bass.Bass("TRN2", target_bir_lowering=False, debug=False)

    xT = nc.dram_tensor("xT", [D, t_core], F16, kind="ExternalInput")
    dT = nc.dram_tensor("dT", [D, NR], F16, kind="ExternalInput")
    uT = nc.dram_tensor("uT", [R, NO], F16, kind="ExternalInput")
    bw = nc.dram_tensor("bw", [1, NO], F16, kind="ExternalInput")
    y = nc.dram_tensor("y", [t_core, NO], F16, kind="ExternalOutput")

    with tile.TileContext(nc) as tc, ExitStack() as ctx:
        const = ctx.enter_context(tc.tile_pool(name="const", bufs=1))
        xpool = ctx.enter_context(tc.tile_pool(name="xpool", bufs=3))
        hpool = ctx.enter_context(tc.tile_pool(name="hpool", bufs=2))
        ypool = ctx.enter_context(tc.tile_pool(name="ypool", bufs=3))
        ps_y = ctx.enter_context(tc.tile_pool(name="ps_y", bufs=3, space="PSUM"))
        ps_h = ctx.enter_context(tc.tile_pool(name="ps_h", bufs=2, space="PSUM"))

        # Resident weights: downT first (phase 1 needs it immediately).
        dT_sb = const.tile([P, DO, NR], F16, name="dT_sb")
        nc.sync.dma_start(dT_sb[:], dT.ap().rearrange("(do di) nr -> di do nr", di=P))

        if PE_WARMUP:
            # ~3.3us of garbage matmuls (identical stationary operand, so
            # LDW_DEDUPE collapses the weight loads) to warm the HAM clock
            # gate while the first x block is still in flight.
            warm = ps_h.tile([P, TB], F32, tag="hps", name="warm")
            for _ in range(16):
                nc.tensor.matmul(
                    warm[:, 0:384], dT_sb[:, 0, 0:P], dT_sb[:, 0, 0:384],
                    start=True, stop=True,
                )

        xTr = xT.ap().rearrange("(do di) t -> di do t", di=P)

        def load_x(tb, pieces=2):
            xts = []
            dq = DO // pieces
            for h in range(pieces):
                xt = xpool.tile([P, dq, TB], F16, tag="xt", name=f"xt{tb}_{h}")
                nc.sync.dma_start(
                    xt[:], xTr[:, h * dq:(h + 1) * dq, tb * TB:(tb + 1) * TB]
                )
                xts.append(xt)
            return xts, dq

        bw_sb = const.tile([P, NO], F16, name="bw_sb")
        if PE_BCAST:
            # Bias lands in partition 0 (24 KB DMA, right after dT), then a
            # rank-1 PE matmul (ones x bias_row) fans it out to PSUM and
            # ScalarE copies it to SBUF — all during the DMA ramp. This
            # keeps ~3 MB of broadcast writes off the SDMA ports and doubles
            # as the HAM warmup. bw_row borrows a ypool rotation slot that is
            # long dead before the first real output tile needs it.
            bw_row = ypool.tile([1, NO], F16, tag="ysb", name="bw_row")
            nc.sync.dma_start(bw_row[0:1, :], bw[:, :])
            ones = const.tile([1, P], F16, name="ones")
            nc.vector.memset(ones[:], 1.0)
            for c in range(NO // 512):
                bps = ps_y.tile([P, 512], F32, tag="yps", name=f"bps{c}")
                nc.tensor.matmul(
                    bps[:], ones[0:1, :], bw_row[0:1, c * 512:(c + 1) * 512],
                    start=True, stop=True,
                )
                nc.scalar.copy(bw_sb[:, c * 512:(c + 1) * 512], bps[:])

        xts, xdq = load_x(0)

        uT_sb = const.tile([P, NO], F16, name="uT_sb")
        nc.sync.dma_start(uT_sb[:], uT[:, :])
        if GPSIMD_BCAST:
            nc.sync.dma_start(bw_sb[0:1, :], bw[:, :])
            from concourse import library_config
            nc.gpsimd.load_library(library_config.mlp)
            cw = NO // BCAST_CHUNKS
            for c in range(BCAST_CHUNKS):
                nc.gpsimd.partition_broadcast(
                    bw_sb[:, c * cw:(c + 1) * cw], bw_sb[0:1, c * cw:(c + 1) * cw]
                )
        elif not PE_BCAST:
            nc.sync.dma_start(bw_sb[:], bw.ap().to_broadcast((P, NO)))

        def emit_ph1_branch(tb, n, bxts, bxdq, hT):
            hps = ps_h.tile([P, TB], F32, tag="hps", name=f"hps{tb}_{n}")
            for dc in range(DO):
                nc.tensor.matmul(
                    hps[:],
                    dT_sb[:, dc, n * R:(n + 1) * R],
                    bxts[dc // bxdq][:, dc % bxdq, :],
                    start=(dc == 0),
                    stop=(dc == DO - 1),
                )
            nc.scalar.copy(hT[:, n, :], hps[:])

        def emit_th(tb, th, hT):
            t0 = tb * TB + th * P
            ysb = ypool.tile([P, NO], F16, tag="ysb", name=f"ysb{tb}_{th}")
            for n in range(NL):
                lhs = hT[:, n, th * P:(th + 1) * P]
                o0 = n * O
                for oc in range(NOC):
                    yps = ps_y.tile([P, OC], F32, tag="yps",
                                    name=f"yps{tb}_{th}_{n}_{oc}")
                    for hh in range(2):
                        c0 = oc * OC + hh * 512
                        nc.tensor.matmul(
                            yps[:, hh * 512:(hh + 1) * 512],
                            lhs,
                            uT_sb[:, o0 + c0: o0 + c0 + 512],
                            start=True,
                            stop=True,
                        )
                    dst = ysb[:, o0 + oc * OC: o0 + (oc + 1) * OC]
                    if oc < ACT_OC:
                        nc.scalar.copy(dst, yps[:])
                    else:
                        nc.vector.tensor_add(
                            dst, yps[:],
                            bw_sb[:, o0 + oc * OC: o0 + (oc + 1) * OC],
                        )
                # Bias for the ScalarE-copied region (fp16 SBUF 2x mode).
                acn = ACT_OC * OC
                nc.vector.tensor_add(
                    ysb[:, o0: o0 + acn],
                    ysb[:, o0: o0 + acn],
                    bw_sb[:, o0: o0 + acn],
                )
                if tb == n_tb - 1:
                    # Last block: store per branch so the final DMA drain
                    # is 1 MB instead of 3 MB.
                    nc.sync.dma_start(
                        y[t0:t0 + P, o0:o0 + O], ysb[:, o0:o0 + O]
                    )
            if tb < n_tb - 1:
                nc.sync.dma_start(y[t0:t0 + P, :], ysb[:])

        # Software pipeline: phase 1 of block tb+1 is interleaved BETWEEN
        # the th-groups of block tb's phase 2, so a store becomes ready
        # every ~12us instead of leaving a ~20us storeless desert at each
        # block transition (where the DMA queue previously ran dry).
        hT_cur = hpool.tile([P, NL, TB], F16, tag="hT", name="hT0")
        for n in range(NL):
            emit_ph1_branch(0, n, xts, xdq, hT_cur)
        for tb in range(n_tb):
            has_next = tb + 1 < n_tb
            if has_next:
                nxts, nxdq = load_x(tb + 1)
                hT_next = hpool.tile([P, NL, TB], F16, tag="hT",
                                     name=f"hT{tb + 1}")
            for th in range(TB // P):
                emit_th(tb, th, hT_cur)
                if has_next and th < NL:
                    emit_ph1_branch(tb + 1, th, nxts, nxdq, hT_next)
            if has_next:
                hT_cur = hT_next

    _wrap_to_json_with_wait_split(nc)
    return nc


def _dedupe_ldweights(bir: dict) -> None:
    """Remove Ldweights whose stationary operand is identical to the
    previous Ldweights in the same block (the PE keeps the loaded weights
    until replaced). Only sync-free Ldweights are dropped."""
    import json as _json

    for fn in bir.get("functions", []):
        for blk in fn.get("blocks", []):
            insts = blk.get("instructions", [])
            out = []
            last_w = None
            for inst in insts:
                if inst.get("opcode") == "Ldweights":
                    key = _json.dumps(inst.get("ins"), sort_keys=True)
                    si = inst.get("sync_info") or {}
                    if (key == last_w and not si.get("on_wait")
                            and not si.get("on_update")):
                        continue
                    last_w = key
                out.append(inst)
            blk["instructions"] = out


def _legalize_wait_counts(bir: dict) -> None:
    """Split multi-wait instructions: this walrus accepts only ONE sync-wait
    per instruction. Excess waits move onto NoOps inserted just before the
    instruction on the same engine — identical blocking semantics."""
    n_new = 0
    for fn in bir.get("functions", []):
        for blk in fn.get("blocks", []):
            insts = blk.get("instructions", [])
            out = []
            for inst in insts:
                si = inst.get("sync_info")
                waits = (si or {}).get("on_wait") or []
                if len(waits) > 1:
                    for w in waits[:-1]:
                        nonlocal_name = f"I-waitsplit-{id(inst)}-{n_new}"
                        n_new += 1
                        out.append({
                            "debug": inst.get("debug", 0),
                            "engine": inst["engine"],
                            "ins": [],
                            "name": nonlocal_name,
                            "opcode": "NoOp",
                            "outs": [],
                            "sync_info": {"on_update": [], "on_wait": [w]},
                        })
                    si["on_wait"] = [waits[-1]]
                out.append(inst)
            blk["instructions"] = out


def _wrap_to_json_with_wait_split(nc) -> None:
    import json as _json

    orig = nc.to_json_bytes

    def patched():
        d = _json.loads(orig())
        if LDW_DEDUPE:
            _dedupe_ldweights(d)
        _legalize_wait_counts(d)
        return _json.dumps(d).encode()

    nc.to_json_bytes = patched


def prep_inputs(x, down, up, bias, scales):
    """Host-side marshalling: transpose + fold scales + cast fp16."""
    x = np.asarray(x, dtype=np.float32)
    down = np.asarray(down, dtype=np.float32)
    up = np.asarray(up, dtype=np.float32)
    bias = np.asarray(bias, dtype=np.float32)
    scales = np.asarray(scales, dtype=np.float32)

    ws = (WEIGHT * scales).astype(np.float32)                       # [NL]
    xTf = np.ascontiguousarray(x.reshape(M, D).T).astype(np.float16)
    dTf = np.ascontiguousarray(
        np.transpose(down, (2, 0, 1)).reshape(D, NR)).astype(np.float16)
    uTf = np.ascontiguousarray(
        np.transpose(up * ws[:, None, None], (2, 0, 1)).reshape(R, NO)
    ).astype(np.float16)
    bwf = np.ascontiguousarray(
        (bias * ws[:, None]).reshape(1, NO)).astype(np.float16)

    in_maps = []
    for c in range(N_CORES):
        in_maps.append({
            "xT": np.ascontiguousarray(xTf[:, c * T:(c + 1) * T]),
            "dT": dTf,
            "uT": uTf,
            "bw": bwf,
        })
    return in_maps


_CACHED_NC = None


def kernel(x, down, up, bias, scales):
    global _CACHED_NC
    from concourse.bass_utils import run_bass_kernel_spmd

    in_maps = prep_inputs(x, down, up, bias, scales)
    if _CACHED_NC is None:
        _CACHED_NC = build_nc(T)
    res = run_bass_kernel_spmd(_CACHED_NC, in_maps, core_ids=list(range(N_CORES)))
    out = np.concatenate([r["y"] for r in res.results], axis=0)
    return out.astype(np.float32).reshape(B, S, NO)
